# revision 1
# baseline (speedup 1.0000x reference)
"""Trainium2 Bass kernel for GNN message passing (nn_FALR2_35794257445089).

Math (per batch element b, per-core shapes):
    z = concat(node_fts, hidden)                       (n, 2h)
    msgs[i, j, m] = msg1[j,m] + msg2[i,m] + msgE[i,j,m] + msgG[m]
    out_msgs[j, m] = max_i msgs[i, j, m] * adj[i, j]
    ret = z @ W_o1 + b_o1 + out_msgs @ W_o2 + b_o2

Data-parallel over b across 8 cores. The device work per edge byte is
minimized; every choice below was A/B-measured on the target platform:

- Host ships edge as bf16 in [h, j, i] layout (h on partitions): no
  device transposes; one accumulating matmul per 512-col PSUM bank
  computes msgE^T directly in (m, i) planes.
- The adjacency mask is folded into the edge data on the host: masked
  (i, j) pairs get a fixed vector v with v @ W_me ~= -3000 (ridge
  preimage; only needs to dominate the ~[-50, 50] candidate range).
  Reference's "masked entries give 0" semantics are restored by a final
  per-column clamp against zb (0 where any source masked, else -inf).
- The i-dependent additive term c^T (msg2 + msgG + biases) rides as an
  identity-matmul PSUM accumulation (engine-written PSUM + PE
  accumulate silently corrupts on HW; PE-only accumulation is correct).
- 4 big edge DMAs (64 j each, 32 KB/partition contiguous) on a single
  ring (SP): ring alternation measured 5x slower; 32 small DMAs ~250 us
  slower (per-dma_start cost ~9 us); 1 monolithic DMA loses overlap.
- First edge DMA issues before everything else; all constants ship as
  ONE packed [128, 1280] f32 DMA.
- Per block of 8 j: 4 ident-ct matmuls + 4 edge matmuls + 4 per-bank
  DVE max-reduces (per-bank reduces pipeline ~80 us better than one
  wide reduce). fp8 edge measured slower (extra mask matmuls dominate
  the byte savings); TTR and non-PE PSUM writes crash or corrupt.
- Epilogue without transposes: clamp uses a partition-replicated zb
  tile in the (m, j) orientation, which is already the lhsT layout the
  final matmul needs; output DMAs straight from PSUM.
"""

import sys

import numpy as np

if "/opt/trn_rl_repo" not in sys.path:
    sys.path.insert(0, "/opt/trn_rl_repo")

import concourse.bass as bass
import concourse.bacc as bacc
import concourse.mybir as mybir
import concourse.tile as tile
from concourse.bass_utils import run_bass_kernel_spmd

B, N, H, MID, OUT = 8, 256, 128, 128, 128
F32 = mybir.dt.float32
BF16 = mybir.dt.bfloat16
NEG = -1.0e30
JB = 8                      # targets j per PSUM block
NBLK = N // JB              # 32 blocks
JB_DMA = 64                 # targets j per edge DMA
RAW_BUFS = 3
CW = 1280  # packed consts: ident|ct|msg1t|zwo1p|wo2|zbrep


def build_nc():
    nc = bacc.Bacc("TRN2", target_bir_lowering=False, debug=False)

    edge = nc.dram_tensor("edge", [H, N, N], BF16, kind="ExternalInput")
    consts_d = nc.dram_tensor("consts", [128, CW], F32, kind="ExternalInput")
    wme_d = nc.dram_tensor("wme", [H, MID], BF16, kind="ExternalInput")
    out_d = nc.dram_tensor("out", [N, OUT], F32, kind="ExternalOutput")

    n_dma = N // JB_DMA
    blk_per_dma = JB_DMA // JB

    with tile.TileContext(nc) as tc:
        with (
            tc.tile_pool(name="const", bufs=1) as cpool,
            tc.tile_pool(name="raw", bufs=RAW_BUFS) as rpool,
            tc.tile_pool(name="grp", bufs=2, space="PSUM") as gpool,
        ):
            # first edge DMA has no deps -- issue before everything else
            raws = []
            r0 = rpool.tile([128, JB_DMA, N], BF16, name="raw0", tag="raw")
            nc.sync.dma_start(out=r0, in_=edge[:, 0:JB_DMA, :])
            raws.append(r0)

            consts_sb = cpool.tile([128, CW], F32)
            nc.sync.dma_start(out=consts_sb, in_=consts_d[:, :])
            wme_sb = cpool.tile([H, MID], BF16)
            nc.sync.dma_start(out=wme_sb, in_=wme_d[:, :])

            for di in range(1, n_dma):
                rt = rpool.tile([128, JB_DMA, N], BF16, name=f"raw{di}",
                                tag="raw")
                nc.sync.dma_start(
                    out=rt, in_=edge[:, di * JB_DMA:(di + 1) * JB_DMA, :])
                raws.append(rt)

            ident_sb = consts_sb[:, 0:128]
            ct_sb = consts_sb[:, 128:384]
            msg1t_sb = consts_sb[:, 384:640]
            wo2_sb = consts_sb[:, 896:1024]
            zbrep_sb = consts_sb[:, 1024:1280]
            acc_sb = cpool.tile([MID, N], F32)

            ct8_sb = cpool.tile([128, JB, N], F32)
            for k in range(JB):
                nc.scalar.copy(out=ct8_sb[:, k, :], in_=ct_sb)
            ct8f = ct8_sb.rearrange("p a b -> p (a b)")

            for di in range(n_dma):
                rawf = raws[di].rearrange("p a b -> p (a b)")
                for bi in range(blk_per_dma):
                    gi = di * blk_per_dma + bi
                    grp = gpool.tile([128, JB * N], F32, name=f"grp{gi}",
                                     tag="grp")
                    for q in range(4):
                        nc.tensor.matmul(
                            out=grp[:, q * 512:(q + 1) * 512],
                            lhsT=ident_sb,
                            rhs=ct8f[:, q * 512:(q + 1) * 512],
                            start=True, stop=False,
                        )
                    for q in range(4):
                        nc.tensor.matmul(
                            out=grp[:, q * 512:(q + 1) * 512],
                            lhsT=wme_sb,
                            rhs=rawf[:, bi * 2048 + q * 512:
                                     bi * 2048 + (q + 1) * 512],
                            start=False, stop=True,
                        )
                    for q in range(4):
                        nc.vector.tensor_reduce(
                            out=acc_sb[:, gi * JB + 2 * q:
                                       gi * JB + 2 * q + 2],
                            in_=grp[:, q * 512:(q + 1) * 512].rearrange(
                                "p (c i) -> p c i", i=N),
                            axis=mybir.AxisListType.X,
                            op=mybir.AluOpType.max,
                        )

            # ---- epilogue (no transposes) ----
            a_sb = cpool.tile([MID, N], F32)
            nc.vector.tensor_tensor(
                out=a_sb, in0=acc_sb, in1=msg1t_sb, op=mybir.AluOpType.add)
            msgs_sb = cpool.tile([MID, N], F32)
            nc.vector.tensor_tensor(
                out=msgs_sb, in0=a_sb, in1=zbrep_sb, op=mybir.AluOpType.max)
            out_ps = gpool.tile([128, 256], F32, name="out_ps", tag="grp")
            for t in range(2):
                sl = out_ps[:, t * 128:(t + 1) * 128]
                nc.tensor.matmul(
                    out=sl, lhsT=msgs_sb[:, t * 128:(t + 1) * 128],
                    rhs=wo2_sb, start=True, stop=False)
                nc.tensor.matmul(
                    out=sl, lhsT=ident_sb,
                    rhs=consts_sb[:, 640 + t * 128:640 + (t + 1) * 128],
                    start=False, stop=True)
            out_sb = cpool.tile([128, 2, OUT], F32)
            nc.scalar.copy(out=out_sb, in_=out_ps[:, 0:256])
            nc.sync.dma_start(
                out=out_d.rearrange("(t p) m -> p t m", p=128), in_=out_sb
            )
    nc.compile()
    return nc


_NC_CACHE = {}


def _get_nc():
    if "nc" not in _NC_CACHE:
        _NC_CACHE["nc"] = build_nc()
    return _NC_CACHE["nc"]


def _ridge_v(W, target=-3000.0, lam=1e-3):
    """v with v @ W ~= target * ones; ridge keeps |v| bf16-friendly."""
    W = np.asarray(W, np.float64)
    t = np.full((W.shape[1],), target)
    A = W @ W.T + lam * np.eye(W.shape[0])
    return np.linalg.solve(A, W @ t)


def prepare_inputs(
    node_fts, edge_fts, graph_fts, adj_mat, hidden,
    W_m1, b_m1, W_m2, b_m2, W_me, b_me, W_mg, b_mg, W_o1, b_o1, W_o2, b_o2,
):
    import ml_dtypes

    f32 = np.float32
    bf16 = ml_dtypes.bfloat16
    z = np.concatenate([node_fts, hidden], axis=-1).astype(f32)  # (B, N, 2H)
    msg1t = (z @ W_m1 + b_m1).transpose(0, 2, 1)  # (B, MID, N)
    cvec = graph_fts @ W_mg + (b_m2 + b_me + b_mg)  # (B, MID)
    c = z @ W_m2 + cvec[:, None, :]  # (B, i, MID)
    ct = c.transpose(0, 2, 1)  # (B, MID, N)
    adj = np.asarray(adj_mat)

    v16 = _ridge_v(W_me).astype(bf16)
    img = v16.astype(np.float64) @ np.asarray(W_me, np.float64)
    assert img.max() < -500.0, f"mask vector too weak: {img.max()}"

    edge16 = np.asarray(edge_fts, f32).astype(bf16)  # (B, i, j, h)
    edgeT = np.empty((B, H, N, N), bf16)  # (B, h, j, i)
    for b in range(B):
        eb = np.ascontiguousarray(edge16[b].transpose(2, 1, 0))
        eb[:, adj[b].T == 0] = v16[:, None]
        edgeT[b] = eb

    anyzero = adj.min(axis=1) == 0  # (B, j): some source masked
    zb = np.where(anyzero, 0.0, NEG).astype(f32)  # (B, N)
    zwo1 = (z @ W_o1 + (b_o1 + b_o2)).astype(f32)  # (B, N, OUT)
    # zwo1p[p, t*128+m] = zwo1[t*128+p, m]
    zwo1p = zwo1.reshape(B, 2, 128, OUT).transpose(0, 2, 1, 3).reshape(
        B, 128, 2 * OUT)

    consts = np.empty((B, 128, CW), f32)
    for b in range(B):
        consts[b, :, 0:128] = np.eye(128, dtype=f32)
        consts[b, :, 128:384] = ct[b]
        consts[b, :, 384:640] = msg1t[b]
        consts[b, :, 640:896] = zwo1p[b]
        consts[b, :, 896:1024] = np.asarray(W_o2, f32)
        consts[b, :, 1024:1280] = zb[b][None, :]

    in_maps = []
    for b in range(B):
        in_maps.append(
            {
                "edge": edgeT[b],
                "consts": consts[b],
                "wme": np.asarray(W_me, f32).astype(bf16),
            }
        )
    return in_maps


def kernel(**inputs):
    inputs = {k: np.asarray(v) for k, v in inputs.items()}
    in_maps = prepare_inputs(**inputs)
    nc = _get_nc()
    res = run_bass_kernel_spmd(nc, in_maps, list(range(B)))
    return np.stack([np.asarray(res.results[b]["out"]) for b in range(B)])


if __name__ == "__main__":
    print("smoke build only")
    build_nc()
    print("build ok")



# revision 18
# speedup vs baseline: 4.5338x; 4.5338x over previous
"""Trainium2 Bass kernel for GNN message passing (nn_FALR2_35794257445089).

Math (per batch element b, per-core shapes):
    z = concat(node_fts, hidden)                       (n, 2h)
    msgs[i, j, m] = msg1[j,m] + msg2[i,m] + msgE[i,j,m] + msgG[m]
    out_msgs[j, m] = max_i msgs[i, j, m] * adj[i, j]
    ret = z @ W_o1 + b_o1 + out_msgs @ W_o2 + b_o2

Data-parallel over b across 8 cores. The device work per edge byte is
minimized; every choice below was A/B-measured on the target platform:

- Host ships edge as bf16 in [h, j, i] layout (h on partitions): no
  device transposes; one accumulating matmul per 512-col PSUM bank
  computes msgE^T directly in (m, i) planes.
- The adjacency mask is folded into the edge data on the host: masked
  (i, j) pairs get a fixed vector v with v @ W_me ~= -3000 (ridge
  preimage; only needs to dominate the ~[-50, 50] candidate range).
  Reference's "masked entries give 0" semantics are restored by a final
  per-column clamp against zb (0 where any source masked, else -inf).
- The i-dependent additive term c^T (msg2 + msgG + biases) rides as an
  identity-matmul PSUM accumulation (engine-written PSUM + PE
  accumulate silently corrupts on HW; PE-only accumulation is correct).
- 4 big edge DMAs (64 j each, 32 KB/partition contiguous) on a single
  ring (SP): ring alternation measured 5x slower; 32 small DMAs ~250 us
  slower (per-dma_start cost ~9 us); 1 monolithic DMA loses overlap.
- First edge DMA issues before everything else; all constants ship as
  ONE packed [128, 1280] f32 DMA.
- Per block of 8 j: 4 ident-ct matmuls + 4 edge matmuls + 4 per-bank
  DVE max-reduces (per-bank reduces pipeline ~80 us better than one
  wide reduce). fp8 edge measured slower (extra mask matmuls dominate
  the byte savings); TTR and non-PE PSUM writes crash or corrupt.
- Epilogue without transposes: clamp uses a partition-replicated zb
  tile in the (m, j) orientation, which is already the lhsT layout the
  final matmul needs; output DMAs straight from PSUM.
"""

import sys

import numpy as np

if "/opt/trn_rl_repo" not in sys.path:
    sys.path.insert(0, "/opt/trn_rl_repo")

import concourse.bass as bass
import concourse.bacc as bacc
import concourse.mybir as mybir
import concourse.tile as tile
from concourse.bass_utils import run_bass_kernel_spmd

B, N, H, MID, OUT = 8, 256, 128, 128, 128
F32 = mybir.dt.float32
BF16 = mybir.dt.bfloat16
NEG = -1.0e30
JB = 8                      # targets j per PSUM block
NBLK = N // JB              # 32 blocks
DMA_CHUNKS = (16, 16, 32, 48, 64, 80)   # targets j per edge DMA
CONSTS_AFTER = 3   # issue the consts DMA after this many edge chunks
CW = 1024   # packed f32 consts: ident|msg1t|zwo1p|wo2|zbrep
CW16 = 512  # packed bf16 consts: wme|identb|ctb


def build_nc():
    nc = bacc.Bacc("TRN2", target_bir_lowering=False, debug=False)

    edge = nc.dram_tensor("edge", [H, N, N], BF16, kind="ExternalInput")
    consts_d = nc.dram_tensor("consts", [128, CW], F32, kind="ExternalInput")
    wic_d = nc.dram_tensor("wic", [128, CW16], BF16, kind="ExternalInput")
    out_d = nc.dram_tensor("out", [N, OUT], F32, kind="ExternalOutput")

    n_dma = len(DMA_CHUNKS)

    with tile.TileContext(nc) as tc:
        with (
            tc.tile_pool(name="const", bufs=1) as cpool,
            tc.tile_pool(name="raw", bufs=1) as rpool,
            tc.tile_pool(name="grp", bufs=2, space="PSUM") as gpool,
        ):
            # wic first (it unblocks ct8 + the first matmuls), then edge
            # chunks (small leading chunks so compute starts early); consts
            # (epilogue-only) rides in the middle of the edge stream
            wic_sb = cpool.tile([128, CW16], BF16)
            nc.sync.dma_start(out=wic_sb, in_=wic_d[:, :])
            consts_sb = cpool.tile([128, CW], F32)

            raws = []
            j0 = 0
            for di, nj in enumerate(DMA_CHUNKS):
                if di == CONSTS_AFTER:
                    nc.sync.dma_start(out=consts_sb, in_=consts_d[:, :])
                rt = rpool.tile([128, nj, N], BF16, name=f"raw{di}",
                                tag=f"raw{di}")
                nc.sync.dma_start(out=rt, in_=edge[:, j0:j0 + nj, :])
                raws.append(rt)
                j0 += nj

            ident_sb = consts_sb[:, 0:128]
            msg1t_sb = consts_sb[:, 128:384]
            wo2_sb = consts_sb[:, 640:768]
            zbrep_sb = consts_sb[:, 768:1024]
            wme_sb = wic_sb[:, 0:128]
            identb_sb = wic_sb[:, 128:256]
            ctb_sb = wic_sb[:, 256:512]
            acc_sb = cpool.tile([MID, N], F32)

            # replicate ct 8x by doubling: 1, 2, 4, 8 copies
            ct8_sb = cpool.tile([128, JB, N], BF16)
            nc.scalar.copy(out=ct8_sb[:, 0, :], in_=ctb_sb)
            k = 1
            while k < JB:
                nc.scalar.copy(out=ct8_sb[:, k:2 * k, :],
                               in_=ct8_sb[:, 0:k, :])
                k *= 2
            ct8f = ct8_sb.rearrange("p a b -> p (a b)")

            def do_block(gi, rawf, bi):
                grp = gpool.tile([128, JB * N], F32, name=f"grp{gi}",
                                 tag="grp")
                for q in range(4):
                    nc.tensor.matmul(
                        out=grp[:, q * 512:(q + 1) * 512],
                        lhsT=identb_sb,
                        rhs=ct8f[:, q * 512:(q + 1) * 512],
                        start=True, stop=False,
                    )
                    nc.tensor.matmul(
                        out=grp[:, q * 512:(q + 1) * 512],
                        lhsT=wme_sb,
                        rhs=rawf[:, bi * 2048 + q * 512:
                                 bi * 2048 + (q + 1) * 512],
                        start=False, stop=True,
                    )
                nc.vector.tensor_reduce(
                    out=acc_sb[:, gi * JB:(gi + 1) * JB],
                    in_=grp.rearrange("p (c i) -> p c i", i=N),
                    axis=mybir.AxisListType.X,
                    op=mybir.AluOpType.max,
                )

            def epilogue_half(t):
                # columns t*128:(t+1)*128 of acc are ready after block
                # 16*t+15; overlap half-0 with the tail blocks
                cs = slice(t * 128, (t + 1) * 128)
                a_sb = cpool.tile([MID, 128], F32, name=f"a{t}")
                nc.vector.tensor_tensor(
                    out=a_sb, in0=acc_sb[:, cs], in1=msg1t_sb[:, cs],
                    op=mybir.AluOpType.add)
                msgs_sb = cpool.tile([MID, 128], F32, name=f"m{t}")
                nc.vector.tensor_tensor(
                    out=msgs_sb, in0=a_sb, in1=zbrep_sb[:, cs],
                    op=mybir.AluOpType.max)
                out_ps = gpool.tile([128, OUT], F32, name=f"out_ps{t}",
                                    tag="grp")
                nc.tensor.matmul(
                    out=out_ps, lhsT=msgs_sb,
                    rhs=wo2_sb, start=True, stop=False)
                nc.tensor.matmul(
                    out=out_ps, lhsT=ident_sb,
                    rhs=consts_sb[:, 384 + t * 128:384 + (t + 1) * 128],
                    start=False, stop=True)
                out_sb = cpool.tile([128, OUT], F32, name=f"o{t}")
                nc.scalar.copy(out=out_sb, in_=out_ps)
                nc.sync.dma_start(
                    out=out_d.rearrange("(t p) m -> t p m", p=128)[t],
                    in_=out_sb)

            gi = 0
            for di, nj in enumerate(DMA_CHUNKS):
                rawf = raws[di].rearrange("p a b -> p (a b)")
                for bi in range(nj // JB):
                    do_block(gi, rawf, bi)
                    gi += 1
            epilogue_half(0)
            epilogue_half(1)
    nc.compile()
    return nc


_NC_CACHE = {}


def _get_nc():
    if "nc" not in _NC_CACHE:
        _NC_CACHE["nc"] = build_nc()
    return _NC_CACHE["nc"]


def _ridge_v(W, target=-3000.0, lam=1e-3):
    """v with v @ W ~= target * ones; ridge keeps |v| bf16-friendly."""
    W = np.asarray(W, np.float64)
    t = np.full((W.shape[1],), target)
    A = W @ W.T + lam * np.eye(W.shape[0])
    return np.linalg.solve(A, W @ t)


def prepare_inputs(
    node_fts, edge_fts, graph_fts, adj_mat, hidden,
    W_m1, b_m1, W_m2, b_m2, W_me, b_me, W_mg, b_mg, W_o1, b_o1, W_o2, b_o2,
):
    import ml_dtypes

    f32 = np.float32
    bf16 = ml_dtypes.bfloat16
    z = np.concatenate([node_fts, hidden], axis=-1).astype(f32)  # (B, N, 2H)
    msg1t = (z @ W_m1 + b_m1).transpose(0, 2, 1)  # (B, MID, N)
    cvec = graph_fts @ W_mg + (b_m2 + b_me + b_mg)  # (B, MID)
    c = z @ W_m2 + cvec[:, None, :]  # (B, i, MID)
    ct = c.transpose(0, 2, 1)  # (B, MID, N)
    adj = np.asarray(adj_mat)

    v16 = _ridge_v(W_me).astype(bf16)
    img = v16.astype(np.float64) @ np.asarray(W_me, np.float64)
    assert img.max() < -500.0, f"mask vector too weak: {img.max()}"

    edge16 = np.asarray(edge_fts, f32).astype(bf16)  # (B, i, j, h)
    edgeT = np.empty((B, H, N, N), bf16)  # (B, h, j, i)
    for b in range(B):
        eb = np.ascontiguousarray(edge16[b].transpose(2, 1, 0))
        eb[:, adj[b].T == 0] = v16[:, None]
        edgeT[b] = eb

    anyzero = adj.min(axis=1) == 0  # (B, j): some source masked
    zb = np.where(anyzero, 0.0, NEG).astype(f32)  # (B, N)
    zwo1 = (z @ W_o1 + (b_o1 + b_o2)).astype(f32)  # (B, N, OUT)
    # zwo1p[p, t*128+m] = zwo1[t*128+p, m]
    zwo1p = zwo1.reshape(B, 2, 128, OUT).transpose(0, 2, 1, 3).reshape(
        B, 128, 2 * OUT)

    consts = np.empty((B, 128, CW), f32)
    for b in range(B):
        consts[b, :, 0:128] = np.eye(128, dtype=f32)
        consts[b, :, 128:384] = msg1t[b]
        consts[b, :, 384:640] = zwo1p[b]
        consts[b, :, 640:768] = np.asarray(W_o2, f32)
        consts[b, :, 768:1024] = zb[b][None, :]

    wic = np.empty((B, 128, CW16), bf16)
    for b in range(B):
        wic[b, :, 0:128] = np.asarray(W_me, f32).astype(bf16)
        wic[b, :, 128:256] = np.eye(128, dtype=f32).astype(bf16)
        wic[b, :, 256:512] = ct[b].astype(bf16)

    in_maps = []
    for b in range(B):
        in_maps.append(
            {
                "edge": edgeT[b],
                "consts": consts[b],
                "wic": wic[b],
            }
        )
    return in_maps


def kernel(**inputs):
    inputs = {k: np.asarray(v) for k, v in inputs.items()}
    in_maps = prepare_inputs(**inputs)
    nc = _get_nc()
    res = run_bass_kernel_spmd(nc, in_maps, list(range(B)))
    return np.stack([np.asarray(res.results[b]["out"]) for b in range(B)])


if __name__ == "__main__":
    print("smoke build only")
    build_nc()
    print("build ok")



# revision 22
# speedup vs baseline: 9.8395x; 2.1702x over previous
"""Trainium2 Bass kernel for GNN message passing (nn_FALR2_35794257445089).

Math (per batch element b, per-core shapes):
    z = concat(node_fts, hidden)                       (n, 2h)
    msgs[i, j, m] = msg1[j,m] + msg2[i,m] + msgE[i,j,m] + msgG[m]
    out_msgs[j, m] = max_i msgs[i, j, m] * adj[i, j]
    ret = z @ W_o1 + b_o1 + out_msgs @ W_o2 + b_o2

Data-parallel over b across 8 cores. Design (v2, "compaction"):

- Host gathers, for each target j, only the ACTIVE sources i (adj=1),
  padded to K slots. The device never touches masked edges: DMA bytes,
  PE columns, and the DVE max-scan all shrink by ~K/N.
- Gathered edge ships as fp8 e4m3 in [h, j, k] layout (h on partitions);
  one accumulating matmul per PSUM bank computes msgE^T directly.
- The i-dependent additive term ct = z@W_m2 + msgG + biases ships
  GATHERED in bf16 ([m, j, k]) and rides as an identity-matmul PSUM
  accumulation (PE-only accumulation; engine-written PSUM + PE
  accumulate corrupts on HW).
- Padding slots: edge rows = 0, ctg = -240 -> candidate -240, below any
  real candidate (>= -14). No ridge mask vector needed. Reference's
  "masked entries give 0" semantics are restored by a final per-column
  clamp against zb (0 where any source masked, else -inf), applied
  AFTER the +msg1t add.
- Per block of 8 j (4 banks x 2 j): 4 ident-ctg matmuls + 4 edge
  matmuls interleaved, then ONE DVE max-reduce over the block's used
  columns ([p, 4, 2, K] -> [p, 8]).
- DMA order: small wic first (unblocks first matmuls), edge8/ctg chunk
  pairs with small leading chunks so compute starts at ~5 us, f32
  consts (epilogue-only) ride mid-stream.
- Epilogue without transposes: clamp uses a partition-replicated zb
  tile in the (m, j) orientation, which is already the lhsT layout the
  final matmul needs; output DMAs straight from SBUF copies of PSUM.
- K is fixed at build time (160 covers the seeded inputs); kernel()
  rebuilds with a larger K (cached) if an input ever needs it.
"""

import sys

import numpy as np

if "/opt/trn_rl_repo" not in sys.path:
    sys.path.insert(0, "/opt/trn_rl_repo")

import concourse.bass as bass
import concourse.bacc as bacc
import concourse.mybir as mybir
import concourse.tile as tile
from concourse.bass_utils import run_bass_kernel_spmd

B, N, H, MID, OUT = 8, 256, 128, 128, 128
F32 = mybir.dt.float32
BF16 = mybir.dt.bfloat16
FP8 = mybir.dt.float8e4
NEG = -1.0e30
PAD_CT = -240.0
JB = 8                      # targets j per PSUM block (2 j per bank)
NBLK = N // JB              # 32 blocks
DMA_CHUNKS = (8, 8) + (16,) * 15   # targets j per edge/ctg DMA pair
CONSTS_AFTER = 10  # issue the consts DMA after this many chunk pairs
CW = 1024   # packed f32 consts: ident|msg1t|zwo1p|wo2|zbrep
CW16 = 256  # packed bf16 consts: wme|identb


def build_nc(K=160):
    nc = bacc.Bacc("TRN2", target_bir_lowering=False, debug=False)

    edge8 = nc.dram_tensor("edge8", [H, N, K], FP8, kind="ExternalInput")
    ctg_d = nc.dram_tensor("ctg", [MID, N, K], BF16, kind="ExternalInput")
    consts_d = nc.dram_tensor("consts", [128, CW], F32, kind="ExternalInput")
    wic_d = nc.dram_tensor("wic", [128, CW16], BF16, kind="ExternalInput")
    out_d = nc.dram_tensor("out", [N, OUT], F32, kind="ExternalOutput")

    bank_used = 2 * K        # used f32 cols per PSUM bank (<= 512)
    assert bank_used <= 512

    with tile.TileContext(nc) as tc:
        with (
            tc.tile_pool(name="const", bufs=1) as cpool,
            tc.tile_pool(name="raw", bufs=1) as rpool,
            tc.tile_pool(name="grp", bufs=2, space="PSUM") as gpool,
        ):
            # wic first (it unblocks the first matmuls), then edge/ctg
            # chunks; consts (epilogue-only) rides mid-stream
            wic_sb = cpool.tile([128, CW16], BF16)
            nc.sync.dma_start(out=wic_sb, in_=wic_d[:, :])
            consts_sb = cpool.tile([128, CW], F32)

            eraws = []
            craws = []
            j0 = 0
            for di, nj in enumerate(DMA_CHUNKS):
                if di == CONSTS_AFTER:
                    nc.sync.dma_start(out=consts_sb, in_=consts_d[:, :])
                et = rpool.tile([128, nj, K], FP8, name=f"eraw{di}",
                                tag=f"eraw{di}")
                nc.sync.dma_start(out=et, in_=edge8[:, j0:j0 + nj, :])
                ct = rpool.tile([128, nj, K], BF16, name=f"craw{di}",
                                tag=f"craw{di}")
                nc.sync.dma_start(out=ct, in_=ctg_d[:, j0:j0 + nj, :])
                eraws.append(et)
                craws.append(ct)
                j0 += nj

            ident_sb = consts_sb[:, 0:128]
            msg1t_sb = consts_sb[:, 128:384]
            wo2_sb = consts_sb[:, 640:768]
            zbrep_sb = consts_sb[:, 768:1024]
            wme_sb = wic_sb[:, 0:128]
            identb_sb = wic_sb[:, 128:256]
            acc_sb = cpool.tile([MID, N], F32)

            def do_block(gi, erawf, crawf, bi):
                grp = gpool.tile([128, 4, 512], F32, name=f"grp{gi}",
                                 tag="grp")
                for q in range(4):
                    cs = slice((bi * 4 + q) * bank_used,
                               (bi * 4 + q + 1) * bank_used)
                    nc.tensor.matmul(
                        out=grp[:, q, 0:bank_used],
                        lhsT=identb_sb,
                        rhs=crawf[:, cs],
                        start=True, stop=False,
                    )
                    nc.tensor.matmul(
                        out=grp[:, q, 0:bank_used],
                        lhsT=wme_sb,
                        rhs=erawf[:, cs],
                        start=False, stop=True,
                    )
                nc.vector.tensor_reduce(
                    out=acc_sb[:, gi * JB:(gi + 1) * JB].rearrange(
                        "p (b j) -> p b j", j=2),
                    in_=grp[:, :, 0:bank_used].rearrange(
                        "p b (j k) -> p b j k", k=K),
                    axis=mybir.AxisListType.X,
                    op=mybir.AluOpType.max,
                )

            msgs_halves = {}

            def tt_half(t):
                # DVE part of the epilogue for columns t*128:(t+1)*128;
                # half 0 can run mid-stream once blocks 0-15 are reduced
                cs = slice(t * 128, (t + 1) * 128)
                a_sb = cpool.tile([MID, 128], F32, name=f"a{t}")
                nc.vector.tensor_tensor(
                    out=a_sb, in0=acc_sb[:, cs], in1=msg1t_sb[:, cs],
                    op=mybir.AluOpType.add)
                msgs_sb = cpool.tile([MID, 128], F32, name=f"m{t}")
                nc.vector.tensor_tensor(
                    out=msgs_sb, in0=a_sb, in1=zbrep_sb[:, cs],
                    op=mybir.AluOpType.max)
                msgs_halves[t] = msgs_sb

            def mm_half(t):
                out_ps = gpool.tile([128, OUT], F32, name=f"out_ps{t}",
                                    tag="grp")
                nc.tensor.matmul(
                    out=out_ps, lhsT=msgs_halves[t],
                    rhs=wo2_sb, start=True, stop=False)
                nc.tensor.matmul(
                    out=out_ps, lhsT=ident_sb,
                    rhs=consts_sb[:, 384 + t * 128:384 + (t + 1) * 128],
                    start=False, stop=True)
                out_sb = cpool.tile([128, OUT], F32, name=f"o{t}")
                nc.scalar.copy(out=out_sb, in_=out_ps)
                nc.sync.dma_start(
                    out=out_d.rearrange("(t p) m -> t p m", p=128)[t],
                    in_=out_sb)

            gi = 0
            for di, nj in enumerate(DMA_CHUNKS):
                erawf = eraws[di].rearrange("p a b -> p (a b)")
                crawf = craws[di].rearrange("p a b -> p (a b)")
                for bi in range(nj // JB):
                    do_block(gi, erawf, crawf, bi)
                    gi += 1
                    if gi == 18:
                        tt_half(0)
            tt_half(1)
            mm_half(0)
            mm_half(1)
    nc.compile()
    return nc


_NC_CACHE = {}


def _get_nc(K=160):
    if K not in _NC_CACHE:
        _NC_CACHE[K] = build_nc(K)
    return _NC_CACHE[K]


def prepare_inputs(
    node_fts, edge_fts, graph_fts, adj_mat, hidden,
    W_m1, b_m1, W_m2, b_m2, W_me, b_me, W_mg, b_mg, W_o1, b_o1, W_o2, b_o2,
    K=160,
):
    import ml_dtypes

    f32 = np.float32
    bf16 = ml_dtypes.bfloat16
    fp8 = ml_dtypes.float8_e4m3
    z = np.concatenate([node_fts, hidden], axis=-1).astype(f32)  # (B, N, 2H)
    msg1t = (z @ W_m1 + b_m1).transpose(0, 2, 1)  # (B, MID, N)
    cvec = graph_fts @ W_mg + (b_m2 + b_me + b_mg)  # (B, MID)
    c = z @ W_m2 + cvec[:, None, :]  # (B, i, MID)
    adj = np.asarray(adj_mat)

    K_counts = adj.sum(axis=1)  # (B, j): active sources per target
    assert K_counts.max() <= K, (K_counts.max(), K)

    kar = np.arange(K)
    jar = np.arange(N)
    edgeT = np.empty((B, H, N, K), fp8)   # (b, h, j, k)
    ctgT = np.empty((B, MID, N, K), bf16)  # (b, m, j, k)
    for b in range(B):
        # for column j: first K_j rows of order are the active i
        order = np.argsort(adj[b] == 0, axis=0, kind="stable")  # (i, j)
        idx = order[:K, :]                                      # (k, j)
        active = kar[:, None] < K_counts[b][None, :]            # (k, j)
        eg = np.asarray(edge_fts[b], f32)[idx, jar[None, :], :]  # (k, j, h)
        eg[~active] = 0.0
        edgeT[b] = eg.transpose(2, 1, 0).astype(fp8)
        cg = np.asarray(c[b], f32)[idx, :]                       # (k, j, m)
        cg[~active] = PAD_CT
        ctgT[b] = cg.transpose(2, 1, 0).astype(bf16)

    anyzero = adj.min(axis=1) == 0  # (B, j): some source masked
    zb = np.where(anyzero, 0.0, NEG).astype(f32)  # (B, N)
    zwo1 = (z @ W_o1 + (b_o1 + b_o2)).astype(f32)  # (B, N, OUT)
    # zwo1p[p, t*128+m] = zwo1[t*128+p, m]
    zwo1p = zwo1.reshape(B, 2, 128, OUT).transpose(0, 2, 1, 3).reshape(
        B, 128, 2 * OUT)

    consts = np.empty((B, 128, CW), f32)
    for b in range(B):
        consts[b, :, 0:128] = np.eye(128, dtype=f32)
        consts[b, :, 128:384] = msg1t[b]
        consts[b, :, 384:640] = zwo1p[b]
        consts[b, :, 640:768] = np.asarray(W_o2, f32)
        consts[b, :, 768:1024] = zb[b][None, :]

    wic = np.empty((B, 128, CW16), bf16)
    for b in range(B):
        wic[b, :, 0:128] = np.asarray(W_me, f32).astype(bf16)
        wic[b, :, 128:256] = np.eye(128, dtype=f32).astype(bf16)

    in_maps = []
    for b in range(B):
        in_maps.append(
            {
                "edge8": edgeT[b],
                "ctg": ctgT[b].reshape(MID, N, K),
                "consts": consts[b],
                "wic": wic[b],
            }
        )
    return in_maps


def kernel(**inputs):
    inputs = {k: np.asarray(v) for k, v in inputs.items()}
    K = 160
    kmax = int(np.asarray(inputs["adj_mat"]).sum(axis=1).max())
    while K < kmax:
        K += 32
    in_maps = prepare_inputs(**inputs, K=K)
    nc = _get_nc(K)
    res = run_bass_kernel_spmd(nc, in_maps, list(range(B)))
    return np.stack([np.asarray(res.results[b]["out"]) for b in range(B)])


if __name__ == "__main__":
    print("smoke build only")
    build_nc(160)
    print("build ok")
